# revision 5
# baseline (speedup 1.0000x reference)
"""Trainium2 Bass kernel for nn_ScoreNet (12-layer GCN message passing + MLP).

8-core SPMD strategy:
- Host: fold symmetric normalization into per-edge weights
  (norm_w = dis[src]*w*dis[dst]); relabel nodes by in-degree rank and shard
  dst nodes round-robin-by-rank across 8 cores; build per-core padded
  K-slab edge structures (per 128-node dst tile: K_t slabs of 128
  (src,weight) pairs).
- Device, per layer (aggregation commutes with the dense transform:
  A(h)W == A(hW), so aggregate FIRST at d_in): per dst tile, gather K_t
  slabs of h[src] via indirect DMA, weighted-sum on DVE,
  p = 2*agg - h_own, then h_new^T = tanh(W^T p^T + b) on PE/ACT,
  transpose back and stage node-major. After all tiles: DMA the shard to
  DRAM and AllGather so every core has full h for the next layer.
- MLP tail + final tanh per node; output staged node-major.
"""

import sys

sys.path.insert(0, "/opt/trn_rl_repo")

import numpy as np

N = 50000
NCORES = 8
P = 128
IN_CH = 32
MP_UNITS = [32, 32, 32, 32, 16, 16, 16, 16, 8, 8, 8, 8]
MLP_UNITS = [16, 16]

Q = (N + NCORES - 1) // NCORES
T = (Q + P - 1) // P                    # 49 tiles per core
SHARD = P * T                           # 6272 padded slots per core
NPAD = NCORES * SHARD                   # 50176


def _preprocess(edge_index, edge_weight):
    src = np.asarray(edge_index[0]).astype(np.int64)
    dst = np.asarray(edge_index[1]).astype(np.int64)
    w = np.asarray(edge_weight).astype(np.float64)

    deg = np.bincount(src, weights=w, minlength=N)
    dis = np.where(deg > 0, 1.0 / np.sqrt(np.maximum(deg, 1e-30)), 0.0)
    nw = (dis[src] * w * dis[dst]).astype(np.float32)

    indeg = np.bincount(dst, minlength=N)
    order = np.argsort(-indeg, kind="stable")

    new_id = np.empty(N, dtype=np.int64)
    r = np.arange(N)
    new_id[order] = (r % NCORES) * SHARD + ((r // NCORES) % P) * T + (r // NCORES) // P

    dst_new = new_id[dst]
    src_new = new_id[src].astype(np.int32)
    dcore = dst_new // SHARD
    drem = dst_new % SHARD
    dpart = drem // T
    dtile = drem % T

    key = (dcore * T + dtile) * P + dpart
    eorder = np.argsort(key, kind="stable")
    src_s = src_new[eorder]
    nw_s = nw[eorder]

    counts = np.bincount(key, minlength=NCORES * T * P).reshape(NCORES, T, P)
    Kt = counts.max(axis=(0, 2)).astype(np.int64)
    koff = np.concatenate([[0], np.cumsum(Kt)]).astype(np.int64)
    Ktot = int(Kt.sum())

    idx_all = np.zeros((NCORES, P, Ktot), dtype=np.int32)
    w_all = np.zeros((NCORES, P, Ktot), dtype=np.float32)
    starts = np.concatenate([[0], np.cumsum(counts.reshape(-1))])
    for ci in range(NCORES):
        for ti in range(T):
            k0 = koff[ti]
            base = (ci * T + ti) * P
            cnt = counts[ci, ti]
            for pi in range(P):
                n = cnt[pi]
                if n:
                    e0 = starts[base + pi]
                    idx_all[ci, pi, k0:k0 + n] = src_s[e0:e0 + n]
                    w_all[ci, pi, k0:k0 + n] = nw_s[e0:e0 + n]
    return new_id, Kt, koff, idx_all, w_all


def _build_program(Kt, koff):
    import concourse.bass as bass
    import concourse.bacc as bacc
    import concourse.mybir as mybir
    import concourse.tile as tile

    f32 = mybir.dt.float32
    i32 = mybir.dt.int32
    AF = mybir.ActivationFunctionType
    Ktot = int(koff[-1])
    Kmax = int(Kt.max())
    dims = [IN_CH] + MP_UNITS
    mdims = [MP_UNITS[-1]] + MLP_UNITS
    nlayers = len(MP_UNITS)

    nc = bacc.Bacc("TRN2", target_bir_lowering=False, num_devices=NCORES)

    xs_ext = nc.declare_dram_parameter("xs", [P, T * IN_CH], f32, isOutput=False)
    x_ext = nc.declare_dram_parameter("x", [NPAD, IN_CH], f32, isOutput=False)
    idx_ext = nc.declare_dram_parameter("idx", [P, Ktot], i32, isOutput=False)
    w_ext = nc.declare_dram_parameter("w", [P, Ktot], f32, isOutput=False)
    eye_ext = nc.declare_dram_parameter("eye", [P, P], f32, isOutput=False)
    Wp, Bp = [], []
    for li, (di, do) in enumerate(zip(dims[:-1], dims[1:])):
        Wp.append(nc.declare_dram_parameter(f"W{li}", [di, do], f32, isOutput=False))
        Bp.append(nc.declare_dram_parameter(f"B{li}", [do, 1], f32, isOutput=False))
    MWp, MBp = [], []
    for li, (di, do) in enumerate(zip(mdims[:-1], mdims[1:])):
        MWp.append(nc.declare_dram_parameter(f"MW{li}", [di, do], f32, isOutput=False))
        MBp.append(nc.declare_dram_parameter(f"MB{li}", [do, 1], f32, isOutput=False))
    FW = nc.declare_dram_parameter("FW", [MLP_UNITS[-1], 1], f32, isOutput=False)
    FB = nc.declare_dram_parameter("FB", [P, 1], f32, isOutput=False)
    out_ext = nc.declare_dram_parameter("out", [P, T], f32, isOutput=True)

    h_full = [x_ext]
    shard_dram = []
    for li in range(nlayers - 1):
        do = dims[li + 1]
        h_full.append(nc.dram_tensor(f"hf{li}", [SHARD * NCORES, do], f32))
        shard_dram.append(nc.dram_tensor(f"hs{li}", [SHARD, do], f32))

    with tile.TileContext(nc) as tc:
        with (
            tc.tile_pool(name="const", bufs=1) as cpool,
            tc.tile_pool(name="gpool", bufs=3) as gpool,
            tc.tile_pool(name="small", bufs=4) as spool,
            tc.tile_pool(name="ps_tr", bufs=2, space="PSUM") as ppool_tr,
            tc.tile_pool(name="ps_mm", bufs=2, space="PSUM") as ppool_mm,
            tc.tile_pool(name="ps_tr2", bufs=2, space="PSUM") as ppool_tr2,
        ):
            idx_t = cpool.tile([P, Ktot], i32, tag="idx")
            nc.sync.dma_start(out=idx_t[:], in_=idx_ext[:])
            w_t = cpool.tile([P, Ktot], f32, tag="w")
            nc.sync.dma_start(out=w_t[:], in_=w_ext[:])
            eye_t = cpool.tile([P, P], f32, tag="eye")
            nc.sync.dma_start(out=eye_t[:], in_=eye_ext[:])

            def load_const(pref, plist, shapes):
                ts = []
                for li, pr in enumerate(plist):
                    t_ = cpool.tile(list(pr.shape), f32, tag=f"{pref}{li}")
                    nc.sync.dma_start(out=t_[:], in_=pr[:])
                    ts.append(t_)
                return ts

            Wt = load_const("Wt", Wp, None)
            Bt = load_const("Bt", Bp, None)
            MWt = load_const("MWt", MWp, None)
            MBt = load_const("MBt", MBp, None)
            FWt = cpool.tile([MLP_UNITS[-1], 1], f32, tag="FWt")
            nc.sync.dma_start(out=FWt[:], in_=FW[:])
            FBt = cpool.tile([P, 1], f32, tag="FBt")
            nc.sync.dma_start(out=FBt[:], in_=FB[:])

            stage0 = cpool.tile([P, T * IN_CH], f32, tag="stage0")
            stage1 = cpool.tile([P, T * IN_CH], f32, tag="stage1")
            stage = [stage0, stage1]
            nc.sync.dma_start(out=stage[0][:], in_=xs_ext[:])

            for li in range(nlayers):
                di, do = dims[li], dims[li + 1]
                stage_prev = stage[li % 2]
                stage_cur = stage[(li + 1) % 2]
                hsrc = h_full[li]
                for t in range(T):
                    K = int(Kt[t])
                    k0 = int(koff[t])
                    G = gpool.tile([P, Kmax * IN_CH], f32, tag="G")
                    for k in range(K):
                        nc.gpsimd.indirect_dma_start(
                            out=G[:, k * di:(k + 1) * di],
                            out_offset=None,
                            in_=hsrc[:],
                            in_offset=bass.IndirectOffsetOnAxis(
                                ap=idx_t[:, k0 + k:k0 + k + 1], axis=0
                            ),
                        )
                    # weighted sum over slabs
                    for k in range(K):
                        nc.vector.tensor_scalar_mul(
                            G[:, k * di:(k + 1) * di],
                            G[:, k * di:(k + 1) * di],
                            w_t[:, k0 + k:k0 + k + 1],
                        )
                    agg = spool.tile([P, di], f32, tag="agg")
                    nc.vector.tensor_reduce(
                        out=agg[:],
                        in_=G[:, :K * di].rearrange("p (k d) -> p d k", d=di),
                        axis=mybir.AxisListType.X,
                        op=mybir.AluOpType.add,
                    )
                    pti = spool.tile([P, di], f32, tag="pti")
                    nc.vector.tensor_scalar_mul(pti[:], agg[:], 2.0)
                    nc.vector.tensor_sub(
                        pti[:], pti[:], stage_prev[:, t * di:(t + 1) * di]
                    )
                    pT_ps = ppool_tr.tile([di, P], f32, tag="pT", space="PSUM")
                    nc.tensor.transpose(out=pT_ps[:], in_=pti[:], identity=eye_t[:])
                    pT_s = spool.tile([di, P], f32, tag="pTs")
                    nc.vector.tensor_copy(out=pT_s[:], in_=pT_ps[:])
                    hT_ps = ppool_mm.tile([do, P], f32, tag="hT", space="PSUM")
                    nc.tensor.matmul(
                        out=hT_ps[:], lhsT=Wt[li][:], rhs=pT_s[:],
                        start=True, stop=True,
                    )
                    hnT = spool.tile([do, P], f32, tag="hnT")
                    nc.scalar.activation(
                        hnT[:], hT_ps[:], AF.Tanh, bias=Bt[li][:], scale=1.0
                    )
                    hn_ps = ppool_tr2.tile([P, do], f32, tag="hn", space="PSUM")
                    nc.tensor.transpose(
                        out=hn_ps[:], in_=hnT[:], identity=eye_t[:do, :do]
                    )
                    nc.vector.tensor_copy(
                        out=stage_cur[:, t * do:(t + 1) * do], in_=hn_ps[:]
                    )
                if li < nlayers - 1:
                    nc.sync.dma_start(
                        out=shard_dram[li][:].rearrange("(p t) d -> p (t d)", p=P),
                        in_=stage_cur[:, :T * do],
                    )
                    nc.gpsimd.collective_compute(
                        "AllGather",
                        mybir.AluOpType.bypass,
                        replica_groups=[list(range(NCORES))],
                        ins=[shard_dram[li][:]],
                        outs=[h_full[li + 1][:]],
                    )

            # MLP tail + final score
            stage_last = stage[nlayers % 2]
            d0 = MP_UNITS[-1]
            out_stage = cpool.tile([P, T], f32, tag="outst")
            for t in range(T):
                pT_ps = ppool_tr.tile([d0, P], f32, tag="pT", space="PSUM")
                nc.tensor.transpose(
                    out=pT_ps[:], in_=stage_last[:, t * d0:(t + 1) * d0],
                    identity=eye_t[:],
                )
                cur = spool.tile([d0, P], f32, tag="pTs")
                nc.vector.tensor_copy(out=cur[:], in_=pT_ps[:])
                for mi, (mdi, mdo) in enumerate(zip(mdims[:-1], mdims[1:])):
                    mm_ps = ppool_mm.tile([mdo, P], f32, tag="hT", space="PSUM")
                    nc.tensor.matmul(
                        out=mm_ps[:], lhsT=MWt[mi][:], rhs=cur[:],
                        start=True, stop=True,
                    )
                    nxt = spool.tile([mdo, P], f32, tag=f"mlp{mi}")
                    nc.scalar.activation(
                        nxt[:], mm_ps[:], AF.Relu, bias=MBt[mi][:], scale=1.0
                    )
                    cur = nxt
                sc_ps = ppool_tr2.tile([P, 1], f32, tag="hn", space="PSUM")
                nc.tensor.matmul(
                    out=sc_ps[:], lhsT=cur[:], rhs=FWt[:], start=True, stop=True
                )
                nc.scalar.activation(
                    out_stage[:, t:t + 1], sc_ps[:], AF.Tanh, bias=FBt[:], scale=1.0
                )
            nc.sync.dma_start(out=out_ext[:], in_=out_stage[:])

    nc.compile()
    return nc


_CACHE = {}


def kernel(**inputs):
    from concourse.bass_utils import run_bass_kernel_spmd

    x = np.asarray(inputs["x"], dtype=np.float32)
    edge_index = np.asarray(inputs["edge_index"])
    edge_weight = np.asarray(inputs["edge_weight"], dtype=np.float32)
    mp_w = [np.asarray(w, np.float32) for w in inputs["mp_weights"]]
    mp_b = [np.asarray(b, np.float32) for b in inputs["mp_biases"]]
    mlp_w = [np.asarray(w, np.float32) for w in inputs["mlp_weights"]]
    mlp_b = [np.asarray(b, np.float32) for b in inputs["mlp_biases"]]
    fin_w = np.asarray(inputs["fin_w"], np.float32)
    fin_b = np.asarray(inputs["fin_b"], np.float32)

    new_id, Kt, koff, idx_all, w_all = _preprocess(edge_index, edge_weight)

    key = (tuple(Kt.tolist()),)
    if key not in _CACHE:
        _CACHE[key] = _build_program(Kt, koff)
    nc = _CACHE[key]

    x_pad = np.zeros((NPAD, IN_CH), dtype=np.float32)
    x_pad[new_id] = x

    base = {
        "x": x_pad,
        "eye": np.eye(P, dtype=np.float32),
        "FW": fin_w.reshape(MLP_UNITS[-1], 1),
        "FB": np.full((P, 1), float(fin_b.reshape(-1)[0]), np.float32),
    }
    for li, (w, b) in enumerate(zip(mp_w, mp_b)):
        base[f"W{li}"] = w
        base[f"B{li}"] = b.reshape(-1, 1)
    for li, (w, b) in enumerate(zip(mlp_w, mlp_b)):
        base[f"MW{li}"] = w
        base[f"MB{li}"] = b.reshape(-1, 1)

    in_maps = []
    for c in range(NCORES):
        m = dict(base)
        m["idx"] = idx_all[c]
        m["w"] = w_all[c]
        # own x shard, node-major staged: xs[p, t*IN_CH+c] = x_pad[cSHARD+p*T+t]
        xs = x_pad[c * SHARD:(c + 1) * SHARD].reshape(P, T * IN_CH)
        m["xs"] = xs
        in_maps.append(m)

    res = run_bass_kernel_spmd(nc, in_maps, list(range(NCORES)))
    global LAST_EXEC_NS
    LAST_EXEC_NS = res.exec_time_ns
    full = np.empty(NPAD, dtype=np.float32)
    for c in range(NCORES):
        full[c * SHARD:(c + 1) * SHARD] = np.asarray(
            res.results[c]["out"], np.float32).reshape(-1)
    out = full[new_id].reshape(N, 1).astype(np.float32)
    return out
